# revision 40
# baseline (speedup 1.0000x reference)
"""AttentionBlock Trainium2 kernel: 8-way batch-parallel over 8 NeuronCores.

Reference computation (per batch element b):
    tokens = x[b].reshape(C, N).T                  # [N, C], N=1024, C=512
    qkv    = tokens @ w_proj + b_proj              # [N, 3*512]
    per head h (8 heads, D=64):
        att  = softmax(q_h @ k_h.T / 8, axis=keys) # [N, N]
        res_h = att @ v_h                          # [N, 64]
    out = res @ w_out + b_out + tokens             # [N, C]
    return out.T.reshape(C, 32, 32)

Kernel strategy (per core, one batch element). The wall is ScalarE's exp
(8.4M elements ~ 68-81us); everything else is engineered to stream around
it without stalling it:
  - qk projection computed transposed: qkT = w_qk.T @ x  -> SBUF [d, tokens]
    (w_proj columns host-permuted so each head-pair's q/k occupy partition
    halves 0-63 / 64-127). Per (pair, key-chunk, query-half) the two K=64
    score matmuls share one [128,2,512] PSUM tile and a single dependency,
    so they issue back-to-back and run concurrently on disjoint PE row
    groups; bufs=2 double-buffers the tile against ScalarE so the exp
    stream never waits on the refill.
  - v projection computed untransposed with a ones column per head; the
    attn@v matmul (M=65) makes the softmax denominator ride the same PSUM
    accumulation. The previous pair's attn@v is front-loaded two
    key-chunks per step so its normalize hides under the current pair's
    exp window.
  - normalize: copy numerator/denominator out of PSUM immediately (frees
    the 4 attn@v banks), DMA-reshape the denominator row across 128
    partitions so the plain reciprocal runs on free-size 16, then
    replicate it across 64 partitions: stride-0 free-dim DMA + GpSimd
    multiplies in steady state (keeps the in-order DVE/sync queues free
    of long waits); K=1 ones-matmul broadcast + DVE multiplies for the
    tail pair (PSUM and PE are idle after the last exp).
  - out projection kc=0..2 partials overlap the tail normalize; residual
    and bias fused on DVE. Scratch matmuls pre-warm the PE's HAM clock
    gate during the input-DMA wait; input DMAs are coarse (descriptors
    stripe across all 16 queues) and emitted in consumption order.
  All matmul operands bf16 (fp32 PSUM accumulation).
"""
import sys
sys.path.insert(0, '/opt/trn_rl_repo')

import numpy as np
import ml_dtypes
from contextlib import ExitStack

B, C, N = 8, 512, 1024
NH, D = 8, 64
INNER = NH * D  # 512
SCALE = D ** -0.5

bf16 = ml_dtypes.bfloat16

_cached_run = None
_cached_nc = None


# ---------------------------------------------------------------- bass kernel
def _build_nc(debug_dumps=False):
    import concourse.bass as bass
    import concourse.tile as tile
    from concourse import bacc, mybir
    from concourse import library_config

    f32 = mybir.dt.float32
    b16 = mybir.dt.bfloat16
    ts = bass.ts

    nc = bacc.Bacc("TRN2", target_bir_lowering=False, debug=False)
    if debug_dumps:
        qkT_dump = nc.dram_tensor("qkT_dump", [128, 8, N], b16,
                                  kind="ExternalOutput").ap()
        v_dump = nc.dram_tensor("v_dump", [128, 8, 8 * 65], b16,
                                kind="ExternalOutput").ap()
        u3A_dump = nc.dram_tensor("u3A_dump", [128, 8, N], b16,
                                  kind="ExternalOutput").ap()
        u3B_dump = nc.dram_tensor("u3B_dump", [128, 8, N], b16,
                                  kind="ExternalOutput").ap()
        resT_dump = nc.dram_tensor("resT_dump", [128, 4, N], b16,
                                   kind="ExternalOutput").ap()

    xb_d = nc.dram_tensor("xb", [C, N], b16, kind="ExternalInput").ap()
    x_d = nc.dram_tensor("x", [C, N], f32, kind="ExternalInput").ap()
    wqk_d = nc.dram_tensor("wqk", [C, 1024], b16, kind="ExternalInput").ap()
    bqk_d = nc.dram_tensor("bqk", [128, 8], f32, kind="ExternalInput").ap()
    wv_d = nc.dram_tensor("wv", [C, 512], b16, kind="ExternalInput").ap()
    bvb_d = nc.dram_tensor("bvb", [128, 512], f32, kind="ExternalInput").ap()
    wo_d = nc.dram_tensor("wo", [INNER, C], b16, kind="ExternalInput").ap()
    bo_d = nc.dram_tensor("bo", [128, 4], f32, kind="ExternalInput").ap()
    out_d = nc.dram_tensor("out", [C, N], f32, kind="ExternalOutput").ap()

    with tile.TileContext(nc) as tc, ExitStack() as ctx:
        sb = ctx.enter_context(tc.tile_pool(name="sb", bufs=1))
        upool = ctx.enter_context(tc.tile_pool(name="up", bufs=1))
        rpool = ctx.enter_context(tc.tile_pool(name="rp", bufs=1))

        # ---- persistent SBUF tensors (inputs split for fine-grained deps)
        # Input DMA strategy: one dma_start stripes its descriptors
        # across all 16 DMA queues, but each dma_start costs ~1.2us of
        # serial issue time, and queue FIFOs process descriptors in enqueue
        # order. So: few coarse DMAs, issued in consumption order (xb and
        # the first wqk columns first); the 2.5MB of tail-only tensors
        # (residual x, w_out) are emitted mid-attention instead.
        xb_sb = sb.tile([128, 4, N], b16)
        wqk_sb = sb.tile([128, 4, 1024], b16)
        wqk_r = wqk_d.rearrange("(kc p) j -> p kc j", p=128)
        nc.sync.dma_start(xb_sb[:], xb_d.rearrange("(kc p) n -> p kc n", p=128))
        nc.sync.dma_start(wqk_sb[:, :, 0:256], wqk_r[:, :, 0:256])
        bqk_sb = sb.tile([128, 8], f32)
        nc.sync.dma_start(bqk_sb[:], bqk_d[:])
        nc.sync.dma_start(wqk_sb[:, :, 256:1024], wqk_r[:, :, 256:1024])
        wv_sb = sb.tile([128, 4, 512], b16)
        nc.sync.dma_start(wv_sb[:], wv_d.rearrange("(kc p) j -> p kc j", p=128))
        bvb_sb = sb.tile([128, 512], f32)
        nc.sync.dma_start(bvb_sb[:], bvb_d[:])
        wo_sb = sb.tile([128, 4, 512], b16)
        bo_sb = sb.tile([128, 4], f32)
        final_sb = sb.tile([128, 4, N], f32)    # [c%128, cchunk, token]

        qkT_sb = sb.tile([128, 8, N], b16)      # [inner%128, qk chunk, token]
        v_sb = sb.tile([128, 8, 8 * 65], b16)   # [token%128, tchunk, h*65+(d|one)]
        v4 = v_sb.rearrange("p t (h w) -> p t h w", w=65)
        resT_sb = sb.tile([128, 4, N], b16)     # [inner%128, pair, token]

        nc.vector.memset(v4[:, :, :, 64], 1.0)  # ones column per head
        ones_sb = sb.tile([65, 64], b16)
        nc.vector.memset(ones_sb[64:65, :], 1.0)  # lhsT for tail bcast matmul
        # pre-warm the PE's HAM clock gate during the input-DMA wait:
        # ~24 back-to-back scratch matmuls keep the array busy >3.4us so
        # the first projection matmuls run at full clock
        warm_sb = sb.tile([128, 256], b16)
        nc.vector.memset(warm_sb[:], 0.0)
        with tc.tile_pool(name="warm", bufs=1, space="PSUM") as wp:
            wps = wp.tile([128, 128], f32)
            for _ in range(36):
                nc.tensor.matmul(wps[:], lhsT=warm_sb[:, 0:128],
                                 rhs=warm_sb[:, 128:256],
                                 start=True, stop=True)

        with tc.tile_pool(name="scp", bufs=1, space="PSUM") as scp:

            def qk_chunk(pool, m, tag="pp", bufs=None):
                ps = pool.tile([128, 2, 512], f32, tag=tag, bufs=bufs,
                               name=f"qk{m}")
                for ih in range(2):
                    for kc in range(4):
                        nc.tensor.matmul(
                            ps[:, ih, :],
                            lhsT=wqk_sb[:, kc, ts(m, 128)],
                            rhs=xb_sb[:, kc, ts(ih, 512)],
                            start=(kc == 0), stop=(kc == 3))
                nc.vector.tensor_scalar_add(
                    qkT_sb[:, m, :], ps.rearrange("p a b -> p (a b)"),
                    bqk_sb[:, m, None])

            def v_chunk(pool, c2):
                ps = pool.tile([128, 2, 512], f32, tag="pp", name=f"v{c2}")
                for half in range(2):
                    tch = 2 * c2 + half
                    for kc in range(4):
                        nc.tensor.matmul(
                            ps[:, half, :],
                            lhsT=xb_sb[:, kc, ts(tch, 128)],
                            rhs=wv_sb[:, kc, :],
                            start=(kc == 0), stop=(kc == 3))
                for half in range(2):
                    nc.vector.tensor_add(
                        v4[:, 2 * c2 + half, :, 0:64],
                        ps[:, half, :].rearrange("p (h d) -> p h d", d=64),
                        bvb_sb.rearrange("p (h d) -> p h d", d=64))

            def scores_jc(t, jc, u2):
                """Per query-half ih, one [128,2,512] PSUM tile holds both
                head-halves' scores. The two K=64 matmuls share a single
                dependency (the WAR on this slot's previous exp), issue
                back-to-back and run concurrently on disjoint PE row groups
                (~512 cycles per ih). bufs=2 double-buffers against ScalarE,
                so the refill fully hides under the alternate exp and
                ScalarE streams continuously."""
                qc, kc = 2 * t, 2 * t + 1
                for ih in range(2):
                    s = scp.tile([128, 2, 512], f32, tag="sc", bufs=2,
                                 name=f"s{t}_{jc}_{ih}")
                    nc.tensor.matmul(
                        s[:, 0, :],
                        lhsT=qkT_sb[0:64, kc, ts(jc, 128)],
                        rhs=qkT_sb[0:64, qc, ts(ih, 512)],
                        start=True, stop=True)
                    nc.tensor.matmul(
                        s[:, 1, :],
                        lhsT=qkT_sb[64:128, kc, ts(jc, 128)],
                        rhs=qkT_sb[64:128, qc, ts(ih, 512)],
                        start=True, stop=True)
                    nc.scalar.activation(
                        u2[:, jc, ih, :, :].rearrange("p a b -> p (a b)"),
                        s.rearrange("p a b -> p (a b)"),
                        mybir.ActivationFunctionType.Exp)

            def attnv_jc(rsp, t, jc, u2, res_pair):
                for half in range(2):
                    h = 2 * t + half
                    res = res_pair[half]
                    for ih in range(2):
                        nc.tensor.matmul(
                            res[:, ih, :],
                            lhsT=v_sb[:, jc, h * 65:h * 65 + 65],
                            rhs=u2[:, jc, ih, half, :],
                            start=(jc == 0), stop=(jc == 7))

            def alloc_res_pair(rsp, t):
                return [rsp.tile([65, 2, 512], f32, tag="res", bufs=2,
                                 name=f"res{2 * t + half}")
                        for half in range(2)]

            def normalize(t, res_pair):
                """res[0:64] * (1/res[64]) -> resT. Numerator and denominator
                are copied out of PSUM immediately (releasing the PSUM slots
                for the next pair's attn@v); the denominator is DMA-reshaped
                across 128 partitions so the plain reciprocal runs on
                free-size 16 instead of 1024, reshaped back, then a stride-0
                free-dim DMA replicates it across 64 partitions for an
                all-SBUF-bf16 multiply. Engine split avoids head-of-line
                blocking: small reshape DMAs + tmpO store on gpsimd, wide
                broadcasts on sync (HW queues), steady-state multiplies on
                gpsimd; the tail pair uses the post-exp-idle ScalarE for the
                PSUM copies and DVE for the multiplies."""
                tail = (t == 3)
                raw = rpool.tile([64, 2, 2, 512], b16, tag="raw", bufs=2,
                                 name=f"raw{t}")
                den_sb = rpool.tile([65, 2, N], f32, tag="den", bufs=2,
                                    name=f"den{t}")
                for half in range(2):
                    if tail and half == 0:
                        # post-exp ScalarE takes the even half so both
                        # halves' PSUM evacuation runs in parallel
                        nc.scalar.activation(
                            raw[:, half, :, :].rearrange("p a b -> p (a b)"),
                            res_pair[half][0:64].rearrange("p a b -> p (a b)"),
                            mybir.ActivationFunctionType.Copy)
                        nc.scalar.activation(
                            den_sb[64:65, half, :],
                            res_pair[half][64:65].rearrange("p a b -> p (a b)"),
                            mybir.ActivationFunctionType.Copy)
                    else:
                        nc.vector.tensor_copy(
                            raw[:, half, :, :].rearrange("p a b -> p (a b)"),
                            res_pair[half][0:64].rearrange("p a b -> p (a b)"))
                        nc.vector.tensor_copy(
                            den_sb[64:65, half, :],
                            res_pair[half][64:65].rearrange("p a b -> p (a b)"))
                denR = rpool.tile([128, 2, 8], f32, tag="denR", bufs=2,
                                  name=f"denR{t}")
                for half in range(2):
                    (nc.scalar if tail else nc.sync).dma_start(
                        denR[:, half, :],
                        den_sb[64:65, half, :]
                        .rearrange("o (p k) -> o p k", k=8))
                recR = rpool.tile([128, 2, 8], b16, tag="recR", bufs=2,
                                  name=f"recR{t}")
                with nc.allow_low_precision(
                        reason="bf16 softmax-denominator reciprocal"):
                    nc.vector.reciprocal(
                        recR.rearrange("p a b -> p (a b)"),
                        denR.rearrange("p a b -> p (a b)"))
                recB = rpool.tile([65, 2, N], b16, tag="recB", bufs=2,
                                  name=f"recB{t}")
                for half in range(2):
                    (nc.scalar if tail else nc.sync).dma_start(
                        recB[64:65, half, :]
                        .rearrange("o (p k) -> o p k", k=8),
                        recR[:, half, :])
                tmpO = rpool.tile([64, N], b16, tag="tmpO", bufs=2,
                                  name=f"tmpO{t}")
                if tail:
                    # the tail pair's res slots are already released by the
                    # casts above: reuse them to broadcast the reciprocal
                    # row with a K=1 ones-matmul on the idle PE (no DMA
                    # hops) and multiply on the idle DVE
                    bcs = [rsp.tile([65, 2, 512], f32, tag="res", bufs=2,
                                    name=f"bc{half}")
                           for half in range(2)]
                    for half in range(2):
                        for ih in range(2):
                            nc.tensor.matmul(
                                bcs[half][0:64, ih, :],
                                lhsT=ones_sb[64:65, :],
                                rhs=recB[64:65, half, ts(ih, 512)],
                                start=True, stop=True)
                    for ih in range(2):
                        nc.vector.tensor_mul(
                            resT_sb[0:64, t, ts(ih, 512)],
                            raw[:, 0, ih, :], bcs[0][0:64, ih, :])
                        nc.vector.tensor_mul(
                            tmpO[:, ts(ih, 512)],
                            raw[:, 1, ih, :], bcs[1][0:64, ih, :])
                else:
                    rbc = rpool.tile([64, 2, 2, 512], b16, tag="rbc", bufs=2,
                                     name=f"rbc{t}")
                    for half in range(2):
                        for ih in range(2):
                            nc.sync.dma_start(
                                rbc[:, half, ih, :],
                                recB[64:65, half, ts(ih, 512)]
                                .unsqueeze(1).broadcast_to((1, 64, 512)))
                    for ih in range(2):
                        nc.gpsimd.tensor_mul(
                            resT_sb[0:64, t, ts(ih, 512)],
                            raw[:, 0, ih, :], rbc[:, 0, ih, :])
                        nc.gpsimd.tensor_mul(
                            tmpO[:, ts(ih, 512)],
                            raw[:, 1, ih, :], rbc[:, 1, ih, :])
                nc.gpsimd.dma_start(resT_sb[64:128, t, :], tmpO[:])

            def alloc_u(t):
                # layout: [p, key-chunk, query-half(ih), head-half, 512]
                return upool.tile([128, 8, 2, 2, 512], b16, tag="U", bufs=2,
                                  name=f"u{t}")

            u_0 = alloc_u(0)
            # ---- phase 1: projections, with pair-0 scores/exp interleaved
            with tc.tile_pool(name="pp", bufs=2, space="PSUM") as pp0:
                qk_chunk(pp0, 0)
                qk_chunk(pp0, 1)
                # one ~1.7us projection chunk per jc step roughly matches
                # the ~2.2us exp pace; ordered by earliest consumer: qk2/3
                # (pair-1 scores), v (pair-0 attn@v), qk4-7 (pairs 2-3)
                rest = [("qk", 2), ("qk", 3), ("v", 0), ("v", 1), ("v", 2),
                        ("v", 3), ("qk", 4), ("qk", 5)]
                for jc in range(8):
                    scores_jc(0, jc, u_0)
                    kind, i = rest[jc]
                    (qk_chunk if kind == "qk" else v_chunk)(pp0, i)

            # ---- phase 2: attention pipeline. The previous pair's attn@v
            # is front-loaded (two key-chunks per early step) so it finishes
            # mid-window and its normalize chain hides under this pair's
            # exp stream instead of blocking the output projection.
            with tc.tile_pool(name="rsp", bufs=1, space="PSUM") as rsp:
                res_pair = alloc_res_pair(rsp, 0)
                u_prev, res_prev = u_0, res_pair
                for t in range(1, 4):
                    u_t = alloc_u(t)
                    res_pair = alloc_res_pair(rsp, t)
                    av = 0
                    for jc in range(8):
                        scores_jc(t, jc, u_t)
                        if t in (1, 2) and jc == 5:
                            # qk chunks 6/7 (only needed by pair-3 scores)
                            # borrow a score-tile slot turn in an
                            # attn@v-free step of pairs 1 and 2
                            qk_chunk(scp, 5 + t, tag="sc", bufs=2)
                        if jc >= 1 and av < 8:
                            attnv_jc(rsp, t - 1, av, u_prev, res_prev)
                            av += 1
                            if av < 8:
                                attnv_jc(rsp, t - 1, av, u_prev, res_prev)
                                av += 1
                    normalize(t - 1, res_prev)
                    if t == 1:
                        # tail-only tensors: issued mid-attention when the
                        # DMA queues are idle
                        nc.sync.dma_start(
                            wo_sb[:],
                            wo_d.rearrange("(kc p) c -> p kc c", p=128))
                        nc.sync.dma_start(bo_sb[:], bo_d[:])
                        nc.sync.dma_start(
                            final_sb[:],
                            x_d.rearrange("(cc p) n -> p cc n", p=128))
                    u_prev, res_prev = u_t, res_pair
                for jc in range(8):
                    attnv_jc(rsp, 3, jc, u_prev, res_prev)
                normalize(3, res_prev)
                if debug_dumps:
                    nc.sync.dma_start(qkT_dump[:], qkT_sb[:])
                    nc.sync.dma_start(v_dump[:], v_sb[:])
                    nc.sync.dma_start(
                        u3A_dump[:],
                        u_prev[:, :, :, 0, :].rearrange("p a b c -> p a (b c)"))
                    nc.sync.dma_start(
                        u3B_dump[:],
                        u_prev[:, :, :, 1, :].rearrange("p a b c -> p a (b c)"))
                    nc.sync.dma_start(resT_dump[:], resT_sb[:])

        # ---- output projection + residual. kc=0..2 partial accumulation for
        # all 4 output chunks overlaps pair-3's normalize chain; only the
        # kc=3 matmuls + residual add + store remain in the tail.
        with tc.tile_pool(name="op", bufs=4, space="PSUM") as op:
            pstiles = []
            for cc in range(4):
                nc.vector.tensor_scalar_add(
                    final_sb[:, cc, :], final_sb[:, cc, :], bo_sb[:, cc, None])
                ps = op.tile([128, 2, 512], f32, tag="op", bufs=4,
                             name=f"o{cc}")
                pstiles.append(ps)
            for kc in range(3):
                for cc in range(4):
                    for ih in range(2):
                        nc.tensor.matmul(
                            pstiles[cc][:, ih, :],
                            lhsT=wo_sb[:, kc, ts(cc, 128)],
                            rhs=resT_sb[:, kc, ts(ih, 512)],
                            start=(kc == 0), stop=False)
            for cc in range(4):
                ps = pstiles[cc]
                for ih in range(2):
                    nc.tensor.matmul(
                        ps[:, ih, :],
                        lhsT=wo_sb[:, 3, ts(cc, 128)],
                        rhs=resT_sb[:, 3, ts(ih, 512)],
                        start=False, stop=True)
                nc.vector.tensor_add(
                    final_sb[:, cc, :], ps.rearrange("p a b -> p (a b)"),
                    final_sb[:, cc, :])
                nc.sync.dma_start(
                    out_d.rearrange("(cc p) n -> p cc n", p=128)[:, cc, :],
                    final_sb[:, cc, :])

    nc.compile()
    return nc


# ------------------------------------------------------------- SPMD dispatch
def _make_spmd_fn(nc, n_cores):
    """bass NEFF runner over axon PJRT WITHOUT buffer donation (donation
    hangs the axon backend)."""
    import jax
    import jax.core
    from jax.sharding import Mesh, PartitionSpec
    from jax.experimental.shard_map import shard_map
    from concourse import mybir
    from concourse.bass2jax import _bass_exec_p, install_neuronx_cc_hook

    install_neuronx_cc_hook()

    partition_name = nc.partition_id_tensor.name if nc.partition_id_tensor else None
    in_names, out_names, out_avals = [], [], []
    for alloc in nc.m.functions[0].allocations:
        if not isinstance(alloc, mybir.MemoryLocationSet):
            continue
        name = alloc.memorylocations[0].name
        if alloc.kind == "ExternalInput":
            if name != partition_name:
                in_names.append(name)
        elif alloc.kind == "ExternalOutput":
            out_names.append(name)
            out_avals.append(jax.core.ShapedArray(
                tuple(alloc.tensor_shape), mybir.dt.np(alloc.dtype)))

    n_params = len(in_names)
    all_in_names = list(in_names) + list(out_names)
    if partition_name is not None:
        all_in_names.append(partition_name)
    zero_outs = [np.zeros(a.shape, a.dtype) for a in out_avals]

    def _body(*args):
        operands = list(args)
        if partition_name is not None:
            from concourse.bass2jax import partition_id_tensor
            operands.append(partition_id_tensor())
        return tuple(_bass_exec_p.bind(
            *operands,
            out_avals=tuple(out_avals),
            in_names=tuple(all_in_names),
            out_names=tuple(out_names),
            lowering_input_output_aliases=(),
            sim_require_finite=True,
            sim_require_nnan=True,
            nc=nc,
        ))

    devices = jax.devices()[:n_cores]
    mesh = Mesh(np.asarray(devices), ("core",))
    sharded = jax.jit(
        shard_map(_body, mesh=mesh,
                  in_specs=(PartitionSpec("core"),) * (n_params + len(out_names)),
                  out_specs=(PartitionSpec("core"),) * len(out_names),
                  check_rep=False),
        keep_unused=True)

    def run(in_maps):
        per_core = [[np.asarray(m[k]) for k in in_names] for m in in_maps]
        concat = [np.concatenate([per_core[c][i] for c in range(n_cores)], axis=0)
                  for i in range(n_params)]
        concat += [np.concatenate([z] * n_cores, axis=0) for z in zero_outs]
        outs = [np.asarray(o) for o in sharded(*concat)]
        results = []
        for c in range(n_cores):
            m = {}
            for i, name in enumerate(out_names):
                rows = out_avals[i].shape[0]
                m[name] = outs[i][c * rows:(c + 1) * rows]
            results.append(m)
        return results

    return run


# ------------------------------------------------------------------ host prep
def _prep_weights(w_proj, b_proj, w_out, b_out):
    # permuted qk columns: chunk m (128 cols): pair t=m//2; m even -> q, odd -> k
    perm = np.empty(1024, np.int64)
    scale = np.empty(1024, np.float32)
    for m in range(8):
        t, is_k = m // 2, m % 2
        for p in range(128):
            h = 2 * t + (1 if p >= 64 else 0)
            d = p % 64
            perm[m * 128 + p] = h * 192 + 64 * is_k + d
            scale[m * 128 + p] = 1.0 if is_k else SCALE
    wqk = (w_proj[:, perm] * scale[None, :]).astype(bf16)
    bqk = (b_proj[perm] * scale).astype(np.float32).reshape(8, 128).T.copy()

    vperm = np.array([(j // 64) * 192 + 128 + (j % 64) for j in range(512)],
                     np.int64)
    wv = w_proj[:, vperm].astype(bf16)
    bvb = np.broadcast_to(b_proj[vperm].astype(np.float32), (128, 512)).copy()

    wo = w_out.astype(bf16)
    bo = b_out.astype(np.float32).reshape(4, 128).T.copy()
    return wqk, bqk, wv, bvb, wo, bo


def kernel(x, w_proj, b_proj, w_out, b_out):
    global _cached_run
    x = np.asarray(x, np.float32)
    w_proj = np.asarray(w_proj, np.float32)
    b_proj = np.asarray(b_proj, np.float32)
    w_out = np.asarray(w_out, np.float32)
    b_out = np.asarray(b_out, np.float32)

    global _cached_nc
    if _cached_run is None:
        nc = _build_nc()
        _cached_nc = nc
        _cached_run = _make_spmd_fn(nc, B)

    wqk, bqk, wv, bvb, wo, bo = _prep_weights(w_proj, b_proj, w_out, b_out)
    in_maps = []
    for b in range(B):
        x2d = np.ascontiguousarray(x[b].reshape(C, N))
        in_maps.append(dict(
            x=x2d, xb=x2d.astype(bf16), wqk=wqk, bqk=bqk,
            wv=wv, bvb=bvb, wo=wo, bo=bo))

    res = _cached_run(in_maps)
    out = np.stack([res[b]["out"].reshape(C, 32, 32) for b in range(B)])
    return out.astype(np.float32)


# revision 41
# speedup vs baseline: 1.0015x; 1.0015x over previous
"""AttentionBlock Trainium2 kernel: 8-way batch-parallel over 8 NeuronCores.

Reference computation (per batch element b):
    tokens = x[b].reshape(C, N).T                  # [N, C], N=1024, C=512
    qkv    = tokens @ w_proj + b_proj              # [N, 3*512]
    per head h (8 heads, D=64):
        att  = softmax(q_h @ k_h.T / 8, axis=keys) # [N, N]
        res_h = att @ v_h                          # [N, 64]
    out = res @ w_out + b_out + tokens             # [N, C]
    return out.T.reshape(C, 32, 32)

Kernel strategy (per core, one batch element). The wall is ScalarE's exp
(8.4M elements ~ 68-81us); everything else is engineered to stream around
it without stalling it:
  - qk projection computed transposed: qkT = w_qk.T @ x  -> SBUF [d, tokens]
    (w_proj columns host-permuted so each head-pair's q/k occupy partition
    halves 0-63 / 64-127). Per (pair, key-chunk, query-half) the two K=64
    score matmuls share one [128,2,512] PSUM tile and a single dependency,
    so they issue back-to-back and run concurrently on disjoint PE row
    groups; bufs=2 double-buffers the tile against ScalarE so the exp
    stream never waits on the refill.
  - v projection computed untransposed with a ones column per head; the
    attn@v matmul (M=65) makes the softmax denominator ride the same PSUM
    accumulation. The previous pair's attn@v is front-loaded two
    key-chunks per step so its normalize hides under the current pair's
    exp window.
  - normalize: copy numerator/denominator out of PSUM immediately (frees
    the 4 attn@v banks), DMA-reshape the denominator row across 128
    partitions so the plain reciprocal runs on free-size 16, then
    replicate it across 64 partitions: stride-0 free-dim DMA + GpSimd
    multiplies in steady state (keeps the in-order DVE/sync queues free
    of long waits); K=1 ones-matmul broadcast + DVE multiplies for the
    tail pair (PSUM and PE are idle after the last exp).
  - out projection kc=0..2 partials overlap the tail normalize; residual
    and bias fused on DVE. Scratch matmuls pre-warm the PE's HAM clock
    gate during the input-DMA wait; input DMAs are coarse (descriptors
    stripe across all 16 queues) and emitted in consumption order.
  All matmul operands bf16 (fp32 PSUM accumulation).
"""
import sys
sys.path.insert(0, '/opt/trn_rl_repo')

import numpy as np
import ml_dtypes
from contextlib import ExitStack

B, C, N = 8, 512, 1024
NH, D = 8, 64
INNER = NH * D  # 512
SCALE = D ** -0.5

bf16 = ml_dtypes.bfloat16

_cached_run = None
_cached_nc = None


# ---------------------------------------------------------------- bass kernel
def _build_nc(debug_dumps=False):
    import concourse.bass as bass
    import concourse.tile as tile
    from concourse import bacc, mybir
    from concourse import library_config

    f32 = mybir.dt.float32
    b16 = mybir.dt.bfloat16
    ts = bass.ts

    nc = bacc.Bacc("TRN2", target_bir_lowering=False, debug=False)
    if debug_dumps:
        qkT_dump = nc.dram_tensor("qkT_dump", [128, 8, N], b16,
                                  kind="ExternalOutput").ap()
        v_dump = nc.dram_tensor("v_dump", [128, 8, 8 * 65], b16,
                                kind="ExternalOutput").ap()
        u3A_dump = nc.dram_tensor("u3A_dump", [128, 8, N], b16,
                                  kind="ExternalOutput").ap()
        u3B_dump = nc.dram_tensor("u3B_dump", [128, 8, N], b16,
                                  kind="ExternalOutput").ap()
        resT_dump = nc.dram_tensor("resT_dump", [128, 4, N], b16,
                                   kind="ExternalOutput").ap()

    xb_d = nc.dram_tensor("xb", [C, N], b16, kind="ExternalInput").ap()
    x_d = nc.dram_tensor("x", [C, N], f32, kind="ExternalInput").ap()
    wqk_d = nc.dram_tensor("wqk", [C, 1024], b16, kind="ExternalInput").ap()
    bqk_d = nc.dram_tensor("bqk", [128, 8], f32, kind="ExternalInput").ap()
    wv_d = nc.dram_tensor("wv", [C, 512], b16, kind="ExternalInput").ap()
    bvb_d = nc.dram_tensor("bvb", [128, 512], f32, kind="ExternalInput").ap()
    wo_d = nc.dram_tensor("wo", [INNER, C], b16, kind="ExternalInput").ap()
    bo_d = nc.dram_tensor("bo", [128, 4], f32, kind="ExternalInput").ap()
    out_d = nc.dram_tensor("out", [C, N], f32, kind="ExternalOutput").ap()

    with tile.TileContext(nc) as tc, ExitStack() as ctx:
        sb = ctx.enter_context(tc.tile_pool(name="sb", bufs=1))
        upool = ctx.enter_context(tc.tile_pool(name="up", bufs=1))
        rpool = ctx.enter_context(tc.tile_pool(name="rp", bufs=1))

        # ---- persistent SBUF tensors (inputs split for fine-grained deps)
        # Input DMA strategy: one dma_start stripes its descriptors
        # across all 16 DMA queues, but each dma_start costs ~1.2us of
        # serial issue time, and queue FIFOs process descriptors in enqueue
        # order. So: few coarse DMAs, issued in consumption order (xb and
        # the first wqk columns first); the 2.5MB of tail-only tensors
        # (residual x, w_out) are emitted mid-attention instead.
        xb_sb = sb.tile([128, 4, N], b16)
        wqk_sb = sb.tile([128, 4, 1024], b16)
        wqk_r = wqk_d.rearrange("(kc p) j -> p kc j", p=128)
        nc.sync.dma_start(xb_sb[:], xb_d.rearrange("(kc p) n -> p kc n", p=128))
        nc.sync.dma_start(wqk_sb[:, :, 0:256], wqk_r[:, :, 0:256])
        bqk_sb = sb.tile([128, 8], f32)
        nc.sync.dma_start(bqk_sb[:], bqk_d[:])
        nc.sync.dma_start(wqk_sb[:, :, 256:1024], wqk_r[:, :, 256:1024])
        wv_sb = sb.tile([128, 4, 512], b16)
        nc.sync.dma_start(wv_sb[:], wv_d.rearrange("(kc p) j -> p kc j", p=128))
        bvb_sb = sb.tile([128, 512], f32)
        nc.sync.dma_start(bvb_sb[:], bvb_d[:])
        wo_sb = sb.tile([128, 4, 512], b16)
        bo_sb = sb.tile([128, 4], f32)
        final_sb = sb.tile([128, 4, N], f32)    # [c%128, cchunk, token]

        qkT_sb = sb.tile([128, 8, N], b16)      # [inner%128, qk chunk, token]
        v_sb = sb.tile([128, 8, 8 * 65], b16)   # [token%128, tchunk, h*65+(d|one)]
        v4 = v_sb.rearrange("p t (h w) -> p t h w", w=65)
        resT_sb = sb.tile([128, 4, N], b16)     # [inner%128, pair, token]

        nc.vector.memset(v4[:, :, :, 64], 1.0)  # ones column per head
        ones_sb = sb.tile([65, 64], b16)
        nc.vector.memset(ones_sb[64:65, :], 1.0)  # lhsT for tail bcast matmul
        # pre-warm the PE's HAM clock gate during the input-DMA wait:
        # ~24 back-to-back scratch matmuls keep the array busy >3.4us so
        # the first projection matmuls run at full clock
        warm_sb = sb.tile([128, 256], b16)
        nc.vector.memset(warm_sb[:], 0.0)
        with tc.tile_pool(name="warm", bufs=1, space="PSUM") as wp:
            wps = wp.tile([128, 128], f32)
            for _ in range(36):
                nc.tensor.matmul(wps[:], lhsT=warm_sb[:, 0:128],
                                 rhs=warm_sb[:, 128:256],
                                 start=True, stop=True)

        with tc.tile_pool(name="scp", bufs=1, space="PSUM") as scp:

            def qk_chunk(pool, m, tag="pp", bufs=None):
                ps = pool.tile([128, 2, 512], f32, tag=tag, bufs=bufs,
                               name=f"qk{m}")
                for ih in range(2):
                    for kc in range(4):
                        nc.tensor.matmul(
                            ps[:, ih, :],
                            lhsT=wqk_sb[:, kc, ts(m, 128)],
                            rhs=xb_sb[:, kc, ts(ih, 512)],
                            start=(kc == 0), stop=(kc == 3))
                nc.vector.tensor_scalar_add(
                    qkT_sb[:, m, :], ps.rearrange("p a b -> p (a b)"),
                    bqk_sb[:, m, None])

            def v_chunk(pool, c2):
                ps = pool.tile([128, 2, 512], f32, tag="pp", name=f"v{c2}")
                for half in range(2):
                    tch = 2 * c2 + half
                    for kc in range(4):
                        nc.tensor.matmul(
                            ps[:, half, :],
                            lhsT=xb_sb[:, kc, ts(tch, 128)],
                            rhs=wv_sb[:, kc, :],
                            start=(kc == 0), stop=(kc == 3))
                for half in range(2):
                    nc.vector.tensor_add(
                        v4[:, 2 * c2 + half, :, 0:64],
                        ps[:, half, :].rearrange("p (h d) -> p h d", d=64),
                        bvb_sb.rearrange("p (h d) -> p h d", d=64))

            def scores_jc(t, jc, u2):
                """Per query-half ih, one [128,2,512] PSUM tile holds both
                head-halves' scores. The two K=64 matmuls share a single
                dependency (the WAR on this slot's previous exp), issue
                back-to-back and run concurrently on disjoint PE row groups
                (~512 cycles per ih). bufs=2 double-buffers against ScalarE,
                so the refill fully hides under the alternate exp and
                ScalarE streams continuously."""
                qc, kc = 2 * t, 2 * t + 1
                for ih in range(2):
                    s = scp.tile([128, 2, 512], f32, tag="sc", bufs=2,
                                 name=f"s{t}_{jc}_{ih}")
                    nc.tensor.matmul(
                        s[:, 0, :],
                        lhsT=qkT_sb[0:64, kc, ts(jc, 128)],
                        rhs=qkT_sb[0:64, qc, ts(ih, 512)],
                        start=True, stop=True)
                    nc.tensor.matmul(
                        s[:, 1, :],
                        lhsT=qkT_sb[64:128, kc, ts(jc, 128)],
                        rhs=qkT_sb[64:128, qc, ts(ih, 512)],
                        start=True, stop=True)
                    nc.scalar.activation(
                        u2[:, jc, ih, :, :].rearrange("p a b -> p (a b)"),
                        s.rearrange("p a b -> p (a b)"),
                        mybir.ActivationFunctionType.Exp)

            def attnv_jc(rsp, t, jc, u2, res_pair):
                for half in range(2):
                    h = 2 * t + half
                    res = res_pair[half]
                    for ih in range(2):
                        nc.tensor.matmul(
                            res[:, ih, :],
                            lhsT=v_sb[:, jc, h * 65:h * 65 + 65],
                            rhs=u2[:, jc, ih, half, :],
                            start=(jc == 0), stop=(jc == 7))

            def alloc_res_pair(rsp, t):
                return [rsp.tile([65, 2, 512], f32, tag="res", bufs=2,
                                 name=f"res{2 * t + half}")
                        for half in range(2)]

            def normalize(t, res_pair):
                """res[0:64] * (1/res[64]) -> resT. Numerator and denominator
                are copied out of PSUM immediately (releasing the PSUM slots
                for the next pair's attn@v); the denominator is DMA-reshaped
                across 128 partitions so the plain reciprocal runs on
                free-size 16 instead of 1024, reshaped back, then a stride-0
                free-dim DMA replicates it across 64 partitions for an
                all-SBUF-bf16 multiply. Engine split avoids head-of-line
                blocking: small reshape DMAs + tmpO store on gpsimd, wide
                broadcasts on sync (HW queues), steady-state multiplies on
                gpsimd; the tail pair uses the post-exp-idle ScalarE for the
                PSUM copies and DVE for the multiplies."""
                tail = (t == 3)
                raw = rpool.tile([64, 2, 2, 512], b16, tag="raw", bufs=2,
                                 name=f"raw{t}")
                den_sb = rpool.tile([65, 2, N], f32, tag="den", bufs=2,
                                    name=f"den{t}")
                for half in range(2):
                    if tail:
                        nc.scalar.activation(
                            raw[:, half, :, :].rearrange("p a b -> p (a b)"),
                            res_pair[half][0:64].rearrange("p a b -> p (a b)"),
                            mybir.ActivationFunctionType.Copy)
                    else:
                        nc.vector.tensor_copy(
                            raw[:, half, :, :].rearrange("p a b -> p (a b)"),
                            res_pair[half][0:64].rearrange("p a b -> p (a b)"))
                    nc.vector.tensor_copy(
                        den_sb[64:65, half, :],
                        res_pair[half][64:65].rearrange("p a b -> p (a b)"))
                denR = rpool.tile([128, 2, 8], f32, tag="denR", bufs=2,
                                  name=f"denR{t}")
                for half in range(2):
                    nc.sync.dma_start(
                        denR[:, half, :],
                        den_sb[64:65, half, :]
                        .rearrange("o (p k) -> o p k", k=8))
                recR = rpool.tile([128, 2, 8], b16, tag="recR", bufs=2,
                                  name=f"recR{t}")
                with nc.allow_low_precision(
                        reason="bf16 softmax-denominator reciprocal"):
                    nc.vector.reciprocal(
                        recR.rearrange("p a b -> p (a b)"),
                        denR.rearrange("p a b -> p (a b)"))
                recB = rpool.tile([65, 2, N], b16, tag="recB", bufs=2,
                                  name=f"recB{t}")
                for half in range(2):
                    nc.sync.dma_start(
                        recB[64:65, half, :]
                        .rearrange("o (p k) -> o p k", k=8),
                        recR[:, half, :])
                tmpO = rpool.tile([64, N], b16, tag="tmpO", bufs=2,
                                  name=f"tmpO{t}")
                if tail:
                    # the tail pair's res slots are already released by the
                    # casts above: reuse them to broadcast the reciprocal
                    # row with a K=1 ones-matmul on the idle PE (no DMA
                    # hops) and multiply on the idle DVE
                    bcs = [rsp.tile([65, 2, 512], f32, tag="res", bufs=2,
                                    name=f"bc{half}")
                           for half in range(2)]
                    for half in range(2):
                        for ih in range(2):
                            nc.tensor.matmul(
                                bcs[half][0:64, ih, :],
                                lhsT=ones_sb[64:65, :],
                                rhs=recB[64:65, half, ts(ih, 512)],
                                start=True, stop=True)
                    for ih in range(2):
                        nc.vector.tensor_mul(
                            resT_sb[0:64, t, ts(ih, 512)],
                            raw[:, 0, ih, :], bcs[0][0:64, ih, :])
                        nc.vector.tensor_mul(
                            tmpO[:, ts(ih, 512)],
                            raw[:, 1, ih, :], bcs[1][0:64, ih, :])
                else:
                    rbc = rpool.tile([64, 2, 2, 512], b16, tag="rbc", bufs=2,
                                     name=f"rbc{t}")
                    for half in range(2):
                        for ih in range(2):
                            nc.sync.dma_start(
                                rbc[:, half, ih, :],
                                recB[64:65, half, ts(ih, 512)]
                                .unsqueeze(1).broadcast_to((1, 64, 512)))
                    for ih in range(2):
                        nc.gpsimd.tensor_mul(
                            resT_sb[0:64, t, ts(ih, 512)],
                            raw[:, 0, ih, :], rbc[:, 0, ih, :])
                        nc.gpsimd.tensor_mul(
                            tmpO[:, ts(ih, 512)],
                            raw[:, 1, ih, :], rbc[:, 1, ih, :])
                nc.gpsimd.dma_start(resT_sb[64:128, t, :], tmpO[:])

            def alloc_u(t):
                # layout: [p, key-chunk, query-half(ih), head-half, 512]
                return upool.tile([128, 8, 2, 2, 512], b16, tag="U", bufs=2,
                                  name=f"u{t}")

            u_0 = alloc_u(0)
            # ---- phase 1: projections, with pair-0 scores/exp interleaved
            with tc.tile_pool(name="pp", bufs=2, space="PSUM") as pp0:
                qk_chunk(pp0, 0)
                qk_chunk(pp0, 1)
                # one ~1.7us projection chunk per jc step roughly matches
                # the ~2.2us exp pace; ordered by earliest consumer: qk2/3
                # (pair-1 scores), v (pair-0 attn@v), qk4-7 (pairs 2-3)
                rest = [("qk", 2), ("qk", 3), ("v", 0), ("v", 1), ("v", 2),
                        ("v", 3), ("qk", 4), ("qk", 5)]
                for jc in range(8):
                    scores_jc(0, jc, u_0)
                    kind, i = rest[jc]
                    (qk_chunk if kind == "qk" else v_chunk)(pp0, i)

            # ---- phase 2: attention pipeline. The previous pair's attn@v
            # is front-loaded (two key-chunks per early step) so it finishes
            # mid-window and its normalize chain hides under this pair's
            # exp stream instead of blocking the output projection.
            with tc.tile_pool(name="rsp", bufs=1, space="PSUM") as rsp:
                res_pair = alloc_res_pair(rsp, 0)
                u_prev, res_prev = u_0, res_pair
                for t in range(1, 4):
                    u_t = alloc_u(t)
                    res_pair = alloc_res_pair(rsp, t)
                    av = 0
                    for jc in range(8):
                        scores_jc(t, jc, u_t)
                        if t == 1 and jc in (5, 6):
                            # qk chunks 6/7 (only needed by pair-3 scores)
                            # borrow a score-tile slot turn in the
                            # attn@v-free steps
                            qk_chunk(scp, 6 + (jc == 6), tag="sc",
                                     bufs=2)
                        if jc >= 1 and av < 8:
                            attnv_jc(rsp, t - 1, av, u_prev, res_prev)
                            av += 1
                            if av < 8:
                                attnv_jc(rsp, t - 1, av, u_prev, res_prev)
                                av += 1
                    normalize(t - 1, res_prev)
                    if t == 1:
                        # tail-only tensors: issued mid-attention when the
                        # DMA queues are idle
                        nc.sync.dma_start(
                            wo_sb[:],
                            wo_d.rearrange("(kc p) c -> p kc c", p=128))
                        nc.sync.dma_start(bo_sb[:], bo_d[:])
                        nc.sync.dma_start(
                            final_sb[:],
                            x_d.rearrange("(cc p) n -> p cc n", p=128))
                    u_prev, res_prev = u_t, res_pair
                for jc in range(8):
                    attnv_jc(rsp, 3, jc, u_prev, res_prev)
                normalize(3, res_prev)
                if debug_dumps:
                    nc.sync.dma_start(qkT_dump[:], qkT_sb[:])
                    nc.sync.dma_start(v_dump[:], v_sb[:])
                    nc.sync.dma_start(
                        u3A_dump[:],
                        u_prev[:, :, :, 0, :].rearrange("p a b c -> p a (b c)"))
                    nc.sync.dma_start(
                        u3B_dump[:],
                        u_prev[:, :, :, 1, :].rearrange("p a b c -> p a (b c)"))
                    nc.sync.dma_start(resT_dump[:], resT_sb[:])

        # ---- output projection + residual. kc=0..2 partial accumulation for
        # all 4 output chunks overlaps pair-3's normalize chain; only the
        # kc=3 matmuls + residual add + store remain in the tail.
        with tc.tile_pool(name="op", bufs=4, space="PSUM") as op:
            pstiles = []
            for cc in range(4):
                nc.vector.tensor_scalar_add(
                    final_sb[:, cc, :], final_sb[:, cc, :], bo_sb[:, cc, None])
                ps = op.tile([128, 2, 512], f32, tag="op", bufs=4,
                             name=f"o{cc}")
                pstiles.append(ps)
            for kc in range(3):
                for cc in range(4):
                    for ih in range(2):
                        nc.tensor.matmul(
                            pstiles[cc][:, ih, :],
                            lhsT=wo_sb[:, kc, ts(cc, 128)],
                            rhs=resT_sb[:, kc, ts(ih, 512)],
                            start=(kc == 0), stop=False)
            for cc in range(4):
                ps = pstiles[cc]
                for ih in range(2):
                    nc.tensor.matmul(
                        ps[:, ih, :],
                        lhsT=wo_sb[:, 3, ts(cc, 128)],
                        rhs=resT_sb[:, 3, ts(ih, 512)],
                        start=False, stop=True)
                nc.vector.tensor_add(
                    final_sb[:, cc, :], ps.rearrange("p a b -> p (a b)"),
                    final_sb[:, cc, :])
                nc.sync.dma_start(
                    out_d.rearrange("(cc p) n -> p cc n", p=128)[:, cc, :],
                    final_sb[:, cc, :])

    nc.compile()
    return nc


# ------------------------------------------------------------- SPMD dispatch
def _make_spmd_fn(nc, n_cores):
    """bass NEFF runner over axon PJRT WITHOUT buffer donation (donation
    hangs the axon backend)."""
    import jax
    import jax.core
    from jax.sharding import Mesh, PartitionSpec
    from jax.experimental.shard_map import shard_map
    from concourse import mybir
    from concourse.bass2jax import _bass_exec_p, install_neuronx_cc_hook

    install_neuronx_cc_hook()

    partition_name = nc.partition_id_tensor.name if nc.partition_id_tensor else None
    in_names, out_names, out_avals = [], [], []
    for alloc in nc.m.functions[0].allocations:
        if not isinstance(alloc, mybir.MemoryLocationSet):
            continue
        name = alloc.memorylocations[0].name
        if alloc.kind == "ExternalInput":
            if name != partition_name:
                in_names.append(name)
        elif alloc.kind == "ExternalOutput":
            out_names.append(name)
            out_avals.append(jax.core.ShapedArray(
                tuple(alloc.tensor_shape), mybir.dt.np(alloc.dtype)))

    n_params = len(in_names)
    all_in_names = list(in_names) + list(out_names)
    if partition_name is not None:
        all_in_names.append(partition_name)
    zero_outs = [np.zeros(a.shape, a.dtype) for a in out_avals]

    def _body(*args):
        operands = list(args)
        if partition_name is not None:
            from concourse.bass2jax import partition_id_tensor
            operands.append(partition_id_tensor())
        return tuple(_bass_exec_p.bind(
            *operands,
            out_avals=tuple(out_avals),
            in_names=tuple(all_in_names),
            out_names=tuple(out_names),
            lowering_input_output_aliases=(),
            sim_require_finite=True,
            sim_require_nnan=True,
            nc=nc,
        ))

    devices = jax.devices()[:n_cores]
    mesh = Mesh(np.asarray(devices), ("core",))
    sharded = jax.jit(
        shard_map(_body, mesh=mesh,
                  in_specs=(PartitionSpec("core"),) * (n_params + len(out_names)),
                  out_specs=(PartitionSpec("core"),) * len(out_names),
                  check_rep=False),
        keep_unused=True)

    def run(in_maps):
        per_core = [[np.asarray(m[k]) for k in in_names] for m in in_maps]
        concat = [np.concatenate([per_core[c][i] for c in range(n_cores)], axis=0)
                  for i in range(n_params)]
        concat += [np.concatenate([z] * n_cores, axis=0) for z in zero_outs]
        outs = [np.asarray(o) for o in sharded(*concat)]
        results = []
        for c in range(n_cores):
            m = {}
            for i, name in enumerate(out_names):
                rows = out_avals[i].shape[0]
                m[name] = outs[i][c * rows:(c + 1) * rows]
            results.append(m)
        return results

    return run


# ------------------------------------------------------------------ host prep
def _prep_weights(w_proj, b_proj, w_out, b_out):
    # permuted qk columns: chunk m (128 cols): pair t=m//2; m even -> q, odd -> k
    perm = np.empty(1024, np.int64)
    scale = np.empty(1024, np.float32)
    for m in range(8):
        t, is_k = m // 2, m % 2
        for p in range(128):
            h = 2 * t + (1 if p >= 64 else 0)
            d = p % 64
            perm[m * 128 + p] = h * 192 + 64 * is_k + d
            scale[m * 128 + p] = 1.0 if is_k else SCALE
    wqk = (w_proj[:, perm] * scale[None, :]).astype(bf16)
    bqk = (b_proj[perm] * scale).astype(np.float32).reshape(8, 128).T.copy()

    vperm = np.array([(j // 64) * 192 + 128 + (j % 64) for j in range(512)],
                     np.int64)
    wv = w_proj[:, vperm].astype(bf16)
    bvb = np.broadcast_to(b_proj[vperm].astype(np.float32), (128, 512)).copy()

    wo = w_out.astype(bf16)
    bo = b_out.astype(np.float32).reshape(4, 128).T.copy()
    return wqk, bqk, wv, bvb, wo, bo


def kernel(x, w_proj, b_proj, w_out, b_out):
    global _cached_run
    x = np.asarray(x, np.float32)
    w_proj = np.asarray(w_proj, np.float32)
    b_proj = np.asarray(b_proj, np.float32)
    w_out = np.asarray(w_out, np.float32)
    b_out = np.asarray(b_out, np.float32)

    global _cached_nc
    if _cached_run is None:
        nc = _build_nc()
        _cached_nc = nc
        _cached_run = _make_spmd_fn(nc, B)

    wqk, bqk, wv, bvb, wo, bo = _prep_weights(w_proj, b_proj, w_out, b_out)
    in_maps = []
    for b in range(B):
        x2d = np.ascontiguousarray(x[b].reshape(C, N))
        in_maps.append(dict(
            x=x2d, xb=x2d.astype(bf16), wqk=wqk, bqk=bqk,
            wv=wv, bvb=bvb, wo=wo, bo=bo))

    res = _cached_run(in_maps)
    out = np.stack([res[b]["out"].reshape(C, 32, 32) for b in range(B)])
    return out.astype(np.float32)
